# revision 3
# baseline (speedup 1.0000x reference)
"""FFT depthwise conv == direct 7x7 circular depthwise conv, on 8 TRN2 cores.

out[b,i,j,c] = sum_{u,v} wf[c,u,v] * x[b,(i+u-3)%H,(j+v-3)%W,c],  wf = kernel[:, ::-1, ::-1]

v1: banded-circulant matmul formulation (vs v0's diagonal matmuls).

Sharding: channel-parallel (24 channels per core, all 8 images). Host lays x
out as (c, w_pad, b, h_pad) bf16 with circular 3-pad on both spatial axes, so
the W axis sits on SBUF partitions and (b, h) is the matmul free axis.

Per (channel, kernel-row a): the 7-tap conv along W is ONE matmul with a
banded Toeplitz stationary A[wp, jl] = wf[c, a, wp-jl] (wp-jl in 0..6), giving
7 useful MACs per contraction column instead of 1 (v0's diagonal trick).
The 7 kernel rows a accumulate into the same PSUM bank by sliding the moving
operand's h-offset (start=a==0, stop=a==6). Per channel: 2 j-tiles of 112
outputs x 4 free-chunks of 448 (b-pairs) = 8 PSUM banks, 7x8 = 56 matmuls of
[128 contraction x 112 out x 448 free] ~= 448 PE cycles each.

Stationary is padded to 128 contraction rows (rows 118..127 are zero weights)
so FWL (fast weight load) triggers; the jt=1 x-tile's tail partitions are
memset once per channel so zero-weight rows never multiply NaN junk.

Engines: PE matmuls; DVE merges PSUM->SBUF bf16; sync-engine HWDGE ring does
all input DMAs (prefetched 2 channels ahead); scalar-engine HWDGE ring does
all output DMAs; gpsimd memsets.
"""

import os
import sys

for _p in ("/opt/trn_rl_repo", "/root/.axon_site/_ro/trn_rl_repo"):
    if os.path.isdir(_p) and _p not in sys.path:
        sys.path.insert(0, _p)

import numpy as np

import concourse.bacc as bacc
import concourse.bass as bass
import concourse.mybir as mybir
from concourse.bass_utils import run_bass_kernel_spmd
from concourse.tile import TileContext

F32 = mybir.dt.float32
BF16 = mybir.dt.bfloat16

B, H, W, C, K = 8, 224, 224, 192, 7
NCORES = 8
CPC = C // NCORES      # 24 channels per core
PAD = K // 2           # 3
PH, PW = H + 2 * PAD, W + 2 * PAD  # 230, 230
JW = 112               # output j-tile width (2 tiles cover W=224)
FREE = B * PH          # 1840 free elems per x partition (b, h_pad)
NCH = 4                # free chunks per j-tile (pairs of images)
CHF = 2 * H            # 448 free elems per chunk (one b-pair)
TAILROWS = PW - JW     # 118 real contraction rows for jt=1


def build_nc():
    nc = bacc.Bacc()
    x_d = nc.declare_dram_parameter("x", [CPC, PW, B, PH], BF16, isOutput=False)
    w_d = nc.declare_dram_parameter("w", [CPC, 128, K, JW], BF16, isOutput=False)
    out_d = nc.declare_dram_parameter("out", [CPC, W, B, H], BF16, isOutput=True)

    xh = x_d.tensor if hasattr(x_d, "tensor") else x_d
    wh = w_d.tensor if hasattr(w_d, "tensor") else w_d
    oh = out_d.tensor if hasattr(out_d, "tensor") else out_d

    mult = mybir.AluOpType.mult

    with TileContext(nc) as tc:
        with (
            tc.tile_pool(name="xin", bufs=6) as xpool,
            tc.tile_pool(name="win", bufs=3) as wpool,
            tc.tile_pool(name="outp", bufs=6) as opool,
            tc.tile_pool(name="psum", bufs=8, space="PSUM") as ppool,
        ):
            loaded = {}

            def load_channel(ci):
                xts = []
                for jt in range(2):
                    xt = xpool.tile([128, B, PH], BF16, name=f"x{ci}_{jt}", tag="x")
                    rows = 128 if jt == 0 else TAILROWS
                    if jt == 1:
                        # zero-weight stationary rows must not see NaN junk;
                        # engine APs need 32-aligned base partition, so clear
                        # 96..127 first and let the DMA overwrite 96..117
                        nc.gpsimd.memset(xt[96:128, :, :], 0.0)
                    src = bass.AP(
                        xh, (ci * PW + jt * JW) * FREE, [[FREE, rows], [1, FREE]]
                    )
                    nc.sync.dma_start(out=xt[0:rows, :, :], in_=src)
                    xts.append(xt)
                wt = wpool.tile([128, K, JW], BF16, name=f"w{ci}", tag="w")
                nc.sync.dma_start(
                    out=wt[:],
                    in_=bass.AP(
                        wh, ci * 128 * K * JW, [[K * JW, 128], [1, K * JW]]
                    ),
                )
                loaded[ci] = (xts, wt)

            load_channel(0)
            load_channel(1)

            for ci in range(CPC):
                if ci + 2 < CPC:
                    load_channel(ci + 2)
                xts, wt = loaded.pop(ci)

                pss = [
                    ppool.tile([JW, 512], F32, name=f"ps{ci}_{k}", tag="ps")
                    for k in range(8)
                ]
                for a in range(K):
                    lhs = wt[:, a, :]
                    for jt in range(2):
                        for ch in range(NCH):
                            rhs = xts[jt][:, 2 * ch : 2 * ch + 2, a : a + H]
                            nc.tensor.matmul(
                                pss[jt * NCH + ch][:, 0:CHF],
                                lhs,
                                rhs,
                                start=(a == 0),
                                stop=(a == K - 1),
                            )

                for jt in range(2):
                    ot = opool.tile([JW, B, H], BF16, name=f"o{ci}_{jt}", tag="o")
                    for ch in range(NCH):
                        ps3 = pss[jt * NCH + ch][:, 0:CHF].rearrange(
                            "p (r w) -> p r w", r=2
                        )
                        nc.vector.tensor_scalar(
                            ot[:, 2 * ch : 2 * ch + 2, :], ps3, 1.0, None, mult
                        )
                    nc.scalar.dma_start(
                        out=bass.AP(
                            oh,
                            (ci * W + jt * JW) * B * H,
                            [[B * H, JW], [1, B * H]],
                        ),
                        in_=ot[:],
                    )
    return nc


def _host_weights(kernel):
    """kernel (C,K,K) -> banded Toeplitz stationaries [C, 128, K, JW] f32."""
    wf = np.ascontiguousarray(kernel[:, ::-1, ::-1]).astype(np.float32)
    wband = np.zeros((C, 128, K, JW), dtype=np.float32)
    jl = np.arange(JW)
    for v in range(K):
        # wband[c, jl+v, a, jl] = wf[c, a, v]
        wband[:, jl + v, :, jl] = wf[:, :, v]
    return wband


_NC_CACHE = {}


def _get_nc():
    if "nc" not in _NC_CACHE:
        nc = build_nc()
        nc.finalize()
        _NC_CACHE["nc"] = nc
    return _NC_CACHE["nc"]


def run(x, kernel, trace=False, **kw):
    import ml_dtypes

    assert x.shape == (B, H, W, C) and kernel.shape == (C, K, K)
    nc = _get_nc()
    # (b, h, w, c) -> (c, w, b, h), circular 3-pad on w and h
    xT = np.transpose(np.asarray(x, dtype=np.float32), (3, 2, 0, 1))
    xTp = np.pad(xT, ((0, 0), (PAD, PAD), (0, 0), (PAD, PAD)), mode="wrap")
    xTp = np.ascontiguousarray(xTp).astype(ml_dtypes.bfloat16)
    wband = _host_weights(np.asarray(kernel)).astype(ml_dtypes.bfloat16)
    in_maps = [
        {
            "x": xTp[core * CPC : (core + 1) * CPC],
            "w": np.ascontiguousarray(wband[core * CPC : (core + 1) * CPC]),
        }
        for core in range(NCORES)
    ]
    res = run_bass_kernel_spmd(nc, in_maps, list(range(NCORES)), trace=trace, **kw)
    # per core out: [CPC, W(j), B, H(i)] -> full [B, H(i), W(j), C]
    arr = np.stack(
        [np.asarray(res.results[core]["out"]) for core in range(NCORES)]
    ).astype(np.float32)
    out = np.ascontiguousarray(np.transpose(arr, (3, 4, 2, 0, 1))).reshape(
        B, H, W, C
    )
    return out, res


def kernel(x, kernel):
    out, _ = run(np.asarray(x), np.asarray(kernel))
    return out


# revision 5
# speedup vs baseline: 1.0070x; 1.0070x over previous
"""FFT depthwise conv == direct 7x7 circular depthwise conv, on 8 TRN2 cores.

out[b,i,j,c] = sum_{u,v} wf[c,u,v] * x[b,(i+u-3)%H,(j+v-3)%W,c],  wf = kernel[:, ::-1, ::-1]

v1: banded-circulant matmul formulation (vs v0's diagonal matmuls).

Sharding: channel-parallel (24 channels per core, all 8 images). Host lays x
out as (c, w_pad, b, h_pad) bf16 with circular 3-pad on both spatial axes, so
the W axis sits on SBUF partitions and (b, h) is the matmul free axis.

Per (channel, kernel-row a): the 7-tap conv along W is ONE matmul with a
banded Toeplitz stationary A[wp, jl] = wf[c, a, wp-jl] (wp-jl in 0..6), giving
7 useful MACs per contraction column instead of 1 (v0's diagonal trick).
The 7 kernel rows a accumulate into the same PSUM bank by sliding the moving
operand's h-offset (start=a==0, stop=a==6). Per channel: 2 j-tiles of 112
outputs x 4 free-chunks of 448 (b-pairs) = 8 PSUM banks, 7x8 = 56 matmuls of
[128 contraction x 112 out x 448 free] ~= 448 PE cycles each.

Stationary is padded to 128 contraction rows (rows 118..127 are zero weights)
so FWL (fast weight load) triggers; the jt=1 x-tile's tail partitions are
memset once per channel so zero-weight rows never multiply NaN junk.

Engines: PE matmuls; DVE merges PSUM->SBUF bf16; sync-engine HWDGE ring does
all input DMAs (prefetched 2 channels ahead); scalar-engine HWDGE ring does
all output DMAs; gpsimd memsets.
"""

import os
import sys

for _p in ("/opt/trn_rl_repo", "/root/.axon_site/_ro/trn_rl_repo"):
    if os.path.isdir(_p) and _p not in sys.path:
        sys.path.insert(0, _p)

import numpy as np

import concourse.bacc as bacc
import concourse.bass as bass
import concourse.mybir as mybir
from concourse.bass_utils import run_bass_kernel_spmd
from concourse.tile import TileContext

F32 = mybir.dt.float32
BF16 = mybir.dt.bfloat16

B, H, W, C, K = 8, 224, 224, 192, 7
NCORES = 8
CPC = C // NCORES      # 24 channels per core
PAD = K // 2           # 3
PH, PW = H + 2 * PAD, W + 2 * PAD  # 230, 230
JW = 112               # output j-tile width (2 tiles cover W=224)
FREE = B * PH          # 1840 free elems per x partition (b, h_pad)
NCH = 4                # free chunks per j-tile (pairs of images)
CHF = 2 * H            # 448 free elems per chunk (one b-pair)
TAILROWS = PW - JW     # 118 real contraction rows for jt=1


def build_nc():
    nc = bacc.Bacc()
    x_d = nc.declare_dram_parameter("x", [CPC, PW, B, PH], BF16, isOutput=False)
    w_d = nc.declare_dram_parameter("w", [CPC, 128, K, JW], BF16, isOutput=False)
    out_d = nc.declare_dram_parameter("out", [CPC, W, B, H], BF16, isOutput=True)

    xh = x_d.tensor if hasattr(x_d, "tensor") else x_d
    wh = w_d.tensor if hasattr(w_d, "tensor") else w_d
    oh = out_d.tensor if hasattr(out_d, "tensor") else out_d

    mult = mybir.AluOpType.mult

    with TileContext(nc) as tc:
        with (
            tc.tile_pool(name="xin", bufs=6) as xpool,
            tc.tile_pool(name="win", bufs=3) as wpool,
            tc.tile_pool(name="outp", bufs=6) as opool,
            tc.tile_pool(name="psum", bufs=8, space="PSUM") as ppool,
        ):
            loaded = {}

            def load_channel(ci, startup=False):
                # during startup the scalar HWDGE ring is idle: split the
                # first channels' loads across both rings to cut time to
                # the first matmul
                eng2 = nc.scalar if startup else nc.sync
                xts = []
                for jt in range(2):
                    xt = xpool.tile([128, B, PH], BF16, name=f"x{ci}_{jt}", tag="x")
                    rows = 128 if jt == 0 else TAILROWS
                    if jt == 1:
                        # zero-weight stationary rows must not see NaN junk;
                        # engine APs need 32-aligned base partition, so clear
                        # 96..127 first and let the DMA overwrite 96..117
                        nc.gpsimd.memset(xt[96:128, :, :], 0.0)
                    src = bass.AP(
                        xh, (ci * PW + jt * JW) * FREE, [[FREE, rows], [1, FREE]]
                    )
                    (nc.sync if jt == 0 else eng2).dma_start(
                        out=xt[0:rows, :, :], in_=src
                    )
                    xts.append(xt)
                wt = wpool.tile([128, K, JW], BF16, name=f"w{ci}", tag="w")
                eng2.dma_start(
                    out=wt[:],
                    in_=bass.AP(
                        wh, ci * 128 * K * JW, [[K * JW, 128], [1, K * JW]]
                    ),
                )
                loaded[ci] = (xts, wt)

            load_channel(0, startup=True)
            load_channel(1, startup=True)

            for ci in range(CPC):
                if ci + 2 < CPC:
                    load_channel(ci + 2)
                xts, wt = loaded.pop(ci)

                # jt outer: jt0's four banks stop and merge while jt1's 28
                # matmuls still stream, so bank recycling for the next
                # channel is off the PE critical path
                for jt in range(2):
                    pss = [
                        ppool.tile([JW, 512], F32, name=f"ps{ci}_{jt}_{ch}", tag="ps")
                        for ch in range(NCH)
                    ]
                    for a in range(K):
                        lhs = wt[:, a, :]
                        for ch in range(NCH):
                            rhs = xts[jt][:, 2 * ch : 2 * ch + 2, a : a + H]
                            nc.tensor.matmul(
                                pss[ch][:, 0:CHF],
                                lhs,
                                rhs,
                                start=(a == 0),
                                stop=(a == K - 1),
                            )

                    ot = opool.tile([JW, B, H], BF16, name=f"o{ci}_{jt}", tag="o")
                    for ch in range(NCH):
                        ps3 = pss[ch][:, 0:CHF].rearrange("p (r w) -> p r w", r=2)
                        nc.vector.tensor_scalar(
                            ot[:, 2 * ch : 2 * ch + 2, :], ps3, 1.0, None, mult
                        )
                    nc.scalar.dma_start(
                        out=bass.AP(
                            oh,
                            (ci * W + jt * JW) * B * H,
                            [[B * H, JW], [1, B * H]],
                        ),
                        in_=ot[:],
                    )
    return nc


def _host_weights(kernel):
    """kernel (C,K,K) -> banded Toeplitz stationaries [C, 128, K, JW] f32."""
    wf = np.ascontiguousarray(kernel[:, ::-1, ::-1]).astype(np.float32)
    wband = np.zeros((C, 128, K, JW), dtype=np.float32)
    jl = np.arange(JW)
    for v in range(K):
        # wband[c, jl+v, a, jl] = wf[c, a, v]
        wband[:, jl + v, :, jl] = wf[:, :, v]
    return wband


_NC_CACHE = {}


def _get_nc():
    if "nc" not in _NC_CACHE:
        nc = build_nc()
        nc.finalize()
        _NC_CACHE["nc"] = nc
    return _NC_CACHE["nc"]


def run(x, kernel, trace=False, **kw):
    import ml_dtypes

    assert x.shape == (B, H, W, C) and kernel.shape == (C, K, K)
    nc = _get_nc()
    # (b, h, w, c) -> (c, w, b, h), circular 3-pad on w and h
    xT = np.transpose(np.asarray(x, dtype=np.float32), (3, 2, 0, 1))
    xTp = np.pad(xT, ((0, 0), (PAD, PAD), (0, 0), (PAD, PAD)), mode="wrap")
    xTp = np.ascontiguousarray(xTp).astype(ml_dtypes.bfloat16)
    wband = _host_weights(np.asarray(kernel)).astype(ml_dtypes.bfloat16)
    in_maps = [
        {
            "x": xTp[core * CPC : (core + 1) * CPC],
            "w": np.ascontiguousarray(wband[core * CPC : (core + 1) * CPC]),
        }
        for core in range(NCORES)
    ]
    res = run_bass_kernel_spmd(nc, in_maps, list(range(NCORES)), trace=trace, **kw)
    # per core out: [CPC, W(j), B, H(i)] -> full [B, H(i), W(j), C]
    arr = np.stack(
        [np.asarray(res.results[core]["out"]) for core in range(NCORES)]
    ).astype(np.float32)
    out = np.ascontiguousarray(np.transpose(arr, (3, 4, 2, 0, 1))).reshape(
        B, H, W, C
    )
    return out, res


def kernel(x, kernel):
    out, _ = run(np.asarray(x), np.asarray(kernel))
    return out


# revision 7
# speedup vs baseline: 1.1764x; 1.1682x over previous
"""FFT depthwise conv == direct 7x7 circular depthwise conv, on 8 TRN2 cores.

out[b,i,j,c] = sum_{u,v} wf[c,u,v] * x[b,(i+u-3)%H,(j+v-3)%W,c],  wf = kernel[:, ::-1, ::-1]

v1: banded-circulant matmul formulation (vs v0's diagonal matmuls).

Sharding: channel-parallel (24 channels per core, all 8 images). Host lays x
out as (c, w_pad, b, h_pad) bf16 with circular 3-pad on both spatial axes, so
the W axis sits on SBUF partitions and (b, h) is the matmul free axis.

Per (channel, kernel-row a): the 7-tap conv along W is ONE matmul with a
banded Toeplitz stationary A[wp, jl] = wf[c, a, wp-jl] (wp-jl in 0..6), giving
7 useful MACs per contraction column instead of 1 (v0's diagonal trick).
The 7 kernel rows a accumulate into the same PSUM bank by sliding the moving
operand's h-offset (start=a==0, stop=a==6). Per channel: 2 j-tiles of 112
outputs x 4 free-chunks of 448 (b-pairs) = 8 PSUM banks, 7x8 = 56 matmuls of
[128 contraction x 112 out x 448 free] ~= 448 PE cycles each.

Stationary is padded to 128 contraction rows (rows 118..127 are zero weights)
so FWL (fast weight load) triggers; the jt=1 x-tile's tail partitions are
memset once per channel so zero-weight rows never multiply NaN junk.

Engines: PE matmuls; DVE merges PSUM->SBUF bf16; sync-engine HWDGE ring does
all input DMAs (prefetched 2 channels ahead); scalar-engine HWDGE ring does
all output DMAs; gpsimd memsets.
"""

import os
import sys

for _p in ("/opt/trn_rl_repo", "/root/.axon_site/_ro/trn_rl_repo"):
    if os.path.isdir(_p) and _p not in sys.path:
        sys.path.insert(0, _p)

import numpy as np

import concourse.bacc as bacc
import concourse.bass as bass
import concourse.mybir as mybir
from concourse.bass_utils import run_bass_kernel_spmd
from concourse.tile import TileContext

F32 = mybir.dt.float32
BF16 = mybir.dt.bfloat16

B, H, W, C, K = 8, 224, 224, 192, 7
NCORES = 8
CPC = C // NCORES      # 24 channels per core
PAD = K // 2           # 3
PH, PW = H + 2 * PAD, W + 2 * PAD  # 230, 230
JW = 112               # output j-tile width (2 tiles cover W=224)
FREE = B * PH          # 1840 free elems per x partition (b, h_pad)
NCH = 4                # free chunks per j-tile (pairs of images)
CHF = 2 * H            # 448 free elems per chunk (one b-pair)
TAILROWS = PW - JW     # 118 real contraction rows for jt=1


def build_nc():
    nc = bacc.Bacc()
    x_d = nc.declare_dram_parameter("x", [CPC, PW, B, PH], BF16, isOutput=False)
    w_d = nc.declare_dram_parameter("w", [CPC, 128, K, JW], BF16, isOutput=False)
    out_d = nc.declare_dram_parameter("out", [CPC, W, B, H], BF16, isOutput=True)

    xh = x_d.tensor if hasattr(x_d, "tensor") else x_d
    wh = w_d.tensor if hasattr(w_d, "tensor") else w_d
    oh = out_d.tensor if hasattr(out_d, "tensor") else out_d

    mult = mybir.AluOpType.mult

    with TileContext(nc) as tc:
        with (
            tc.tile_pool(name="xin", bufs=6) as xpool,
            tc.tile_pool(name="win", bufs=3) as wpool,
            tc.tile_pool(name="outp", bufs=6) as opool,
            tc.tile_pool(name="psum", bufs=8, space="PSUM") as ppool,
        ):
            loaded = {}

            def load_channel(ci, startup=False):
                # one big DMA's descriptors land on ~2 of the 16 SDMA
                # engines (~23 GB/s each); chunking each load into 4
                # instructions spreads them.  jt=0 chunks ride the sync
                # HWDGE ring, jt=1 + weights the scalar ring.
                xts = []
                for jt in range(2):
                    xt = xpool.tile([128, B, PH], BF16, name=f"x{ci}_{jt}", tag="x")
                    rows = 128 if jt == 0 else TAILROWS
                    if jt == 1:
                        # zero-weight stationary rows must not see NaN junk;
                        # engine APs need 32-aligned base partition, so clear
                        # 96..127 first and let the DMA overwrite 96..117
                        nc.gpsimd.memset(xt[96:128, :, :], 0.0)
                    eng = nc.sync if jt == 0 else nc.scalar
                    base = (ci * PW + jt * JW) * FREE
                    for p0 in range(0, rows, 32):
                        nr = min(32, rows - p0)
                        src = bass.AP(
                            xh, base + p0 * FREE, [[FREE, nr], [1, FREE]]
                        )
                        eng.dma_start(out=xt[p0 : p0 + nr, :, :], in_=src)
                    xts.append(xt)
                wt = wpool.tile([128, K, JW], BF16, name=f"w{ci}", tag="w")
                for p0 in range(0, 128, 64):
                    nc.scalar.dma_start(
                        out=wt[p0 : p0 + 64, :, :],
                        in_=bass.AP(
                            wh,
                            (ci * 128 + p0) * K * JW,
                            [[K * JW, 64], [1, K * JW]],
                        ),
                    )
                loaded[ci] = (xts, wt)

            load_channel(0)
            load_channel(1)

            for ci in range(CPC):
                if ci + 2 < CPC:
                    load_channel(ci + 2)
                xts, wt = loaded.pop(ci)

                # jt outer: jt0's four banks stop and merge while jt1's 28
                # matmuls still stream, so bank recycling for the next
                # channel is off the PE critical path
                for jt in range(2):
                    pss = [
                        ppool.tile([JW, 512], F32, name=f"ps{ci}_{jt}_{ch}", tag="ps")
                        for ch in range(NCH)
                    ]
                    for a in range(K):
                        lhs = wt[:, a, :]
                        for ch in range(NCH):
                            rhs = xts[jt][:, 2 * ch : 2 * ch + 2, a : a + H]
                            nc.tensor.matmul(
                                pss[ch][:, 0:CHF],
                                lhs,
                                rhs,
                                start=(a == 0),
                                stop=(a == K - 1),
                            )

                    ot = opool.tile([JW, B, H], BF16, name=f"o{ci}_{jt}", tag="o")
                    for ch in range(NCH):
                        ps3 = pss[ch][:, 0:CHF].rearrange("p (r w) -> p r w", r=2)
                        nc.vector.tensor_scalar(
                            ot[:, 2 * ch : 2 * ch + 2, :], ps3, 1.0, None, mult
                        )
                    nc.scalar.dma_start(
                        out=bass.AP(
                            oh,
                            (ci * W + jt * JW) * B * H,
                            [[B * H, JW], [1, B * H]],
                        ),
                        in_=ot[:],
                    )
    return nc


def _host_weights(kernel):
    """kernel (C,K,K) -> banded Toeplitz stationaries [C, 128, K, JW] f32."""
    wf = np.ascontiguousarray(kernel[:, ::-1, ::-1]).astype(np.float32)
    wband = np.zeros((C, 128, K, JW), dtype=np.float32)
    jl = np.arange(JW)
    for v in range(K):
        # wband[c, jl+v, a, jl] = wf[c, a, v]
        wband[:, jl + v, :, jl] = wf[:, :, v]
    return wband


_NC_CACHE = {}


def _get_nc():
    if "nc" not in _NC_CACHE:
        nc = build_nc()
        nc.finalize()
        _NC_CACHE["nc"] = nc
    return _NC_CACHE["nc"]


def run(x, kernel, trace=False, **kw):
    import ml_dtypes

    assert x.shape == (B, H, W, C) and kernel.shape == (C, K, K)
    nc = _get_nc()
    # (b, h, w, c) -> (c, w, b, h), circular 3-pad on w and h
    xT = np.transpose(np.asarray(x, dtype=np.float32), (3, 2, 0, 1))
    xTp = np.pad(xT, ((0, 0), (PAD, PAD), (0, 0), (PAD, PAD)), mode="wrap")
    xTp = np.ascontiguousarray(xTp).astype(ml_dtypes.bfloat16)
    wband = _host_weights(np.asarray(kernel)).astype(ml_dtypes.bfloat16)
    in_maps = [
        {
            "x": xTp[core * CPC : (core + 1) * CPC],
            "w": np.ascontiguousarray(wband[core * CPC : (core + 1) * CPC]),
        }
        for core in range(NCORES)
    ]
    res = run_bass_kernel_spmd(nc, in_maps, list(range(NCORES)), trace=trace, **kw)
    # per core out: [CPC, W(j), B, H(i)] -> full [B, H(i), W(j), C]
    arr = np.stack(
        [np.asarray(res.results[core]["out"]) for core in range(NCORES)]
    ).astype(np.float32)
    out = np.ascontiguousarray(np.transpose(arr, (3, 4, 2, 0, 1))).reshape(
        B, H, W, C
    )
    return out, res


def kernel(x, kernel):
    out, _ = run(np.asarray(x), np.asarray(kernel))
    return out
